# revision 73
# baseline (speedup 1.0000x reference)
"""GAT message-passing kernel for 8 trn2 NeuronCores (v3).

Math (reference, per t):
    Wx = x @ W;  s1 = Wx@a1/sqrt(2D);  s2 = Wx@a2/sqrt(2D)
    weight = softmax_src(lrelu(s1[src] + s2[dest]));  agg = lrelu(weight @ Wx)
    out = x - agg

Identities:
    With t = exp(0.99 s1), F1 = exp(0.01 s1), r = exp(-0.99 s2) and a
    per-dest rescale (softmax-invariant), the score-exp tile factors as
        et[src,dest] = F1[src] * max(t[src], r[dest]).
    Folding F1 into the matmul rhs (wxp = [F1*Wx | F1]) leaves two
    engine-friendly tile forms:
        DVE : max(t[src], r)            (1-op tensor_scalar, 4x mode)
        ACT : relu(r - t[src])          (activation, bias=-t)
    relu-form tiles are missing the t[src] part; the difference is exactly
    c = sum_{relu-built src} t[src]*wxp[src,:], injected per accumulator
    with one rank-1 matmul (ones x cvec) closing each PSUM group.
    The softmax denominator rides as wxp's 129th column (= F1).

Schedule: pass q over dest chunk of 512; projections/cvec interleave into
pass 0's PE emission so the PE never drains. Score tiles are built
double-width ([128,1024], one per (pair, mt)) and consumed by 2 passes.
PSUM: 2 proj groups + 1 rb slot + 2x2 acc-pair banks = 8 banks.
"""

import sys

if "/opt/trn_rl_repo" not in sys.path:
    sys.path.insert(0, "/opt/trn_rl_repo")

import numpy as np

N, T, D = 4096, 4, 128
P = 128
HALF = N // 2            # 2048 dest nodes per core
MT = N // P              # 32 src tiles
NT = HALF // P           # 16 dest chunks
SCALE_INV = 1.0 / 16.0   # 1/sqrt(2*128)
PG = 3                   # proj mts per PSUM bank ([128, 3*129] f32)
NPG = (MT + PG - 1) // PG
PJW = PG * (D + 1)       # proj group width (387)

_CACHE = {}


def _tile_assignment():
    """Engine per double-tile build: DVE (max-form) vs ACT (relu-form)."""

    def rr(n, w):
        cnt = {e: 0 for e in w}
        out = []
        for _ in range(n):
            e = min(w, key=lambda k: (cnt[k] + 1) / w[k])
            cnt[e] += 1
            out.append(e)
        return out

    return rr(MT, {"d": 26.0, "a": 6.0}) + rr(MT, {"d": 21.0, "a": 11.0})


def _build():
    import concourse.mybir as mybir
    from concourse import bacc
    from concourse.tile import TileContext

    f32 = mybir.dt.float32
    bf16 = mybir.dt.bfloat16
    fp8 = mybir.dt.float8e4
    Alu = mybir.AluOpType
    Act = mybir.ActivationFunctionType

    nc = bacc.Bacc()
    prm_d = nc.declare_dram_parameter("prm", [P, 2 * D + 2], bf16, isOutput=False)
    xt_d = nc.declare_dram_parameter("xt", [P, N], fp8, isOutput=False)
    xn_d = nc.declare_dram_parameter("xn", [P, NT * D], f32, isOutput=False)
    out = nc.declare_dram_parameter("out", [P, NT * D], f32, isOutput=True)

    assign = _tile_assignment()

    with TileContext(nc) as tc:
        with (
            tc.tile_pool(name="const", bufs=1) as cpool,
            tc.tile_pool(name="fpool", bufs=6) as fpool,
            tc.tile_pool(name="opool", bufs=12) as opool,
        ):
            # ---------------- input DMAs ----------------
            xt = cpool.tile([P, N], fp8)
            prm = cpool.tile([P, 2 * D + 2], bf16)
            xn_sb = cpool.tile([P, NT * D], f32)
            nc.sync.dma_start(xt[:, 0:512], xt_d[:, 0:512])
            nc.scalar.dma_start(prm[:, :], prm_d[:, :])
            nc.sync.dma_start(xt[:, 512:1536], xt_d[:, 512:1536])
            nc.scalar.dma_start(xt[:, 1536:4096], xt_d[:, 1536:4096])
            nc.sync.dma_start(xn_sb[:, :], xn_d[:, :])
            W_sb = prm[:, 0:D]
            WT_sb = prm[:, D : 2 * D]
            av_sb = prm[:, 2 * D : 2 * D + 2]

            # persistent SBUF state
            wproj = cpool.tile([P, D + 1], bf16)       # [W | w1s]
            w2b = cpool.tile([P, P], bf16)
            r_b = cpool.tile([P, HALF], bf16)
            t_a = cpool.tile([P, MT], f32)
            nt_a = cpool.tile([P, MT], f32)            # -t
            tb_a = cpool.tile([P, MT], bf16)           # t in bf16 (cvec lhsT)
            F1a = cpool.tile([P, MT], f32)
            wxp = cpool.tile([P, MT * (D + 1)], bf16)  # [F1*Wx | F1] per mt
            # c-vector duplicated side by side so ONE rank-1 matmul can
            # initialize both accumulators sharing a PSUM bank
            cvec_sb = [
                cpool.tile([1, 2 * (D + 1)], bf16, name=f"cvec{pr}")
                for pr in range(2)
            ]
            ones_1p = cpool.tile([1, P], bf16)
            nc.vector.memset(ones_1p[:, :], 1.0)
            warm_src = cpool.tile([P, 2 * P], bf16)
            nc.vector.memset(warm_src[:, :], 0.001)
            dt = [
                cpool.tile([P, 1024], bf16, name=f"dt{i}") for i in range(2 * MT)
            ]

            with tc.tile_pool(name="ppsum", bufs=3, space="PSUM") as ppool, \
                 tc.tile_pool(name="rbsum", bufs=1, space="PSUM") as rpool, \
                 tc.tile_pool(name="mpsum", bufs=2, space="PSUM") as mpool:
                # ---- PE warm-up: ~4.5us of full-width matmuls so the HAM
                # clock gate sees real array activity and reaches 8/8
                # before the real matmuls start. Output is thrown away.
                wm = mpool.tile([P, 2 * (D + 1)], f32, tag="accA", name="wm")
                for k in range(24):
                    nc.tensor.matmul(
                        wm[:, 0:P],
                        warm_src[:, 0:P],
                        warm_src[:, P : 2 * P],
                        start=True,
                        stop=True,
                    )

                # ---- wproj = [W | (W@a1)/16]; w2col = (W@a2)/16 ----
                nc.vector.tensor_copy(wproj[:, 0:D], W_sb)
                w_ps = ppool.tile([P, PJW], f32, tag="pj", name="w_ps")
                nc.tensor.matmul(
                    w_ps[:, 0:2], WT_sb, av_sb, start=True, stop=True
                )
                nc.scalar.activation(
                    wproj[:, D : D + 1], w_ps[:, 0:1], Act.Copy, scale=SCALE_INV
                )
                sc2 = cpool.tile([P, 1], f32)
                nc.scalar.activation(sc2[:, :], w_ps[:, 1:2], Act.Copy, scale=SCALE_INV)
                nc.vector.tensor_scalar(
                    w2b[:, :], W_sb, 0.0, sc2[:, :], Alu.mult, Alu.add
                )

                # ---- emitters ----
                def emit_rb(q):
                    rb_ps = rpool.tile([P, 512], f32, tag="rb", name=f"rb{q}")
                    nc.tensor.matmul(
                        rb_ps[:, :],
                        w2b[:, :],
                        xt[:, q * 512 : (q + 1) * 512],
                        start=True,
                        stop=True,
                    )
                    nc.scalar.activation(
                        r_b[:, q * 512 : (q + 1) * 512], rb_ps[:, :], Act.Exp,
                        scale=-0.99,
                    )

                def emit_group(g):
                    mts = list(range(g * PG, min((g + 1) * PG, MT)))
                    w = len(mts)
                    p_ps = ppool.tile([P, PJW], f32, tag="pj", name=f"pj{g}")
                    for i, mt in enumerate(mts):
                        nc.tensor.matmul(
                            p_ps[:, i * (D + 1) : (i + 1) * (D + 1)],
                            xt[:, mt * P : (mt + 1) * P],
                            wproj[:, :],
                            start=True,
                            stop=True,
                        )
                    s1v = p_ps[:, D : w * (D + 1) : D + 1]
                    m0 = mts[0]
                    nc.scalar.activation(
                        F1a[:, m0 : m0 + w], s1v, Act.Exp, scale=0.01
                    )
                    nc.scalar.activation(
                        t_a[:, m0 : m0 + w], s1v, Act.Exp, scale=0.99
                    )
                    nc.gpsimd.tensor_scalar(
                        nt_a[:, m0 : m0 + w], t_a[:, m0 : m0 + w],
                        -1.0, None, Alu.mult,
                    )
                    for i, mt in enumerate(mts):
                        nc.vector.tensor_scalar(
                            wxp[:, mt * (D + 1) : mt * (D + 1) + D],
                            p_ps[:, i * (D + 1) : i * (D + 1) + D],
                            F1a[:, mt : mt + 1],
                            None,
                            Alu.mult,
                        )
                    # denominator col (strided dst over this group's blocks)
                    nc.vector.tensor_copy(
                        wxp[:, m0 * (D + 1) + D : (m0 + w) * (D + 1) : D + 1],
                        F1a[:, m0 : m0 + w],
                    )

                def build_tile(idx):
                    pair, mt = idx // MT, idx % MT
                    dst = dt[idx][:, :]
                    src = r_b[:, pair * 1024 : (pair + 1) * 1024]
                    if assign[idx] == "d":
                        nc.vector.tensor_scalar(
                            dst, src, t_a[:, mt : mt + 1], None, Alu.max
                        )
                    else:
                        nc.scalar.activation(
                            dst, src, Act.Relu, bias=nt_a[:, mt : mt + 1]
                        )

                cursor = [0]

                def ensure_built(upto):
                    while cursor[0] <= min(upto, 2 * MT - 1):
                        build_tile(cursor[0])
                        cursor[0] += 1

                def emit_cvec(pr):
                    amts = [m for m in range(MT) if assign[pr * MT + m] == "a"]
                    c_ps = rpool.tile([P, 512], f32, tag="rb", name=f"cv{pr}")
                    for k, mt in enumerate(amts):
                        nc.tensor.matmul(
                            c_ps[0:1, 0 : D + 1],
                            tb_a[:, mt : mt + 1],
                            wxp[:, mt * (D + 1) : (mt + 1) * (D + 1)],
                            start=(k == 0),
                            stop=(k == len(amts) - 1),
                        )
                    nc.vector.tensor_copy(
                        cvec_sb[pr][:, 0 : D + 1], c_ps[0:1, 0 : D + 1]
                    )
                    nc.vector.tensor_copy(
                        cvec_sb[pr][:, D + 1 : 2 * (D + 1)], c_ps[0:1, 0 : D + 1]
                    )

                def finalize_unit(q, acc, j):
                    ndc = q * 4 + j
                    col = (j % 2) * (D + 1)
                    ap = acc[j // 2]
                    rz = fpool.tile([P, 1], f32, tag="rz", name="rz")
                    nc.vector.reciprocal(rz[:, :], ap[:, col + D : col + D + 1])
                    lr = fpool.tile([P, D], f32, tag="lr", name="lr")
                    nc.scalar.activation(
                        lr[:, :], ap[:, col : col + D], Act.Lrelu,
                        scale=rz[:, :], alpha=0.01,
                    )
                    o = opool.tile([P, D], f32, tag="o", name="o")
                    sub_eng = nc.vector if q == 3 else nc.gpsimd
                    sub_eng.tensor_tensor(
                        o[:, :], xn_sb[:, ndc * D : (ndc + 1) * D],
                        lr[:, :], Alu.subtract,
                    )
                    eng = nc.sync if j % 2 == 0 else nc.scalar
                    eng.dma_start(out[:, ndc * D : (ndc + 1) * D], o[:, :])

                def finalize(q, acc):
                    for j in range(4):
                        finalize_unit(q, acc, j)

                # ---- passes ----
                gi = [0]

                def need_groups(n):
                    while gi[0] < min(n, NPG):
                        emit_group(gi[0])
                        gi[0] += 1

                # (q, mt) emission schedule: pass 0, then passes 1+2
                # interleaved 2:1 (pair-1 tiles stream in while the PE chews
                # pass-1 work), then pass 3.
                sched = [(0, mt) for mt in range(MT)]
                sched += [(1, mt) for mt in range(10)]
                mt1, mt2 = 10, 0
                while mt1 < MT:
                    sched.append((1, mt1))
                    mt1 += 1
                    if mt1 < MT:
                        sched.append((1, mt1))
                        mt1 += 1
                    sched.append((2, mt2))
                    mt2 += 1
                sched += [(2, mt) for mt in range(mt2, MT)]
                sched += [(3, mt) for mt in range(MT)]

                accs = {}
                pending = None
                for q, mt in sched:
                    pair = q // 2
                    off = (q % 2) * 512
                    ti = pair * MT + mt
                    if mt == 0:
                        accs[q] = [
                            mpool.tile(
                                [P, 2 * (D + 1)], f32, tag=t, name=f"{t}_{q}"
                            )
                            for t in ("accA", "accB")
                        ]
                    acc = accs[q]
                    if q == 0:
                        # interleave projection production into pass 0
                        if mt == 0:
                            emit_rb(0)
                            need_groups(3)
                            emit_rb(1)
                        # small warm fillers bridge the DMA/proj-chain gaps
                        # so a full HAM window completes early (2.4 GHz from
                        # the start of the main loop instead of ~19us in)
                        if 1 <= mt <= 10:
                            for _ in range(2):
                                nc.tensor.matmul(
                                    wm[0:64, 0:64],
                                    warm_src[:, 0:64],
                                    warm_src[:, P : P + 64],
                                    start=True,
                                    stop=True,
                                )
                        need_groups((mt + 9) // PG + 1)
                        # pair-1 r_b halves aren't needed until the merged
                        # 1+2 phase — keep them out of ACT's early queue
                        if mt == 16:
                            emit_rb(2)
                        if mt == 20:
                            emit_rb(3)
                        if mt == 26:
                            nc.gpsimd.tensor_copy(tb_a[:, :], t_a[:, :])
                        if mt == 28:
                            emit_cvec(0)
                        ensure_built(mt + 8)
                    elif q == 1:
                        if mt == 0:
                            emit_cvec(1)
                    elif q == 2:
                        ensure_built(MT + mt + 14)
                        if mt == 11:
                            # bridge the pass-1-exhaustion lull so the HAM
                            # MID window never sees enough idle to rethrottle
                            wm2 = rpool.tile(
                                [P, 512], f32, tag="rb", name="wm2"
                            )
                            for _ in range(6):
                                nc.tensor.matmul(
                                    wm2[0:64, 0:64],
                                    warm_src[:, 0:64],
                                    warm_src[:, P : P + 64],
                                    start=True,
                                    stop=True,
                                )
                    else:
                        ensure_built(2 * MT - 1)
                    if q == 1 and mt == 6 and pending is not None:
                        finalize(*pending)
                        pending = None
                    for j in range(4):
                        # start=True clears has_written for the WHOLE bank —
                        # only the first MM of each acc-pair bank may set it
                        # (the j-odd half then lands on cleared bits and
                        # overwrites, which is right).
                        nc.tensor.matmul(
                            acc[j // 2][
                                :, (j % 2) * (D + 1) : (j % 2 + 1) * (D + 1)
                            ],
                            dt[ti][:, off + j * P : off + (j + 1) * P],
                            wxp[:, mt * (D + 1) : (mt + 1) * (D + 1)],
                            start=(mt == 0 and j % 2 == 0),
                            stop=False,
                            skip_group_check=(j % 2 == 1),
                        )
                    if mt == MT - 1:
                        # one rank-1 init per acc-pair bank covers both halves
                        for h in range(2):
                            nc.tensor.matmul(
                                acc[h][:, :],
                                ones_1p[:, :],
                                cvec_sb[pair][:, :],
                                start=False,
                                stop=True,
                                skip_group_check=True,
                            )
                            if q == 3:
                                finalize_unit(q, acc, 2 * h)
                                finalize_unit(q, acc, 2 * h + 1)
                        if q in (1, 2):
                            # closes mid-stream / at merged-phase end;
                            # finalize right away so the Lrelu/sub/DMA chain
                            # overlaps later PE work instead of the tail
                            finalize(q, acc)
                        elif q == 0:
                            pending = (q, acc)

    nc.compile()
    return nc


def _prep_inputs(x, W, a1, a2):
    """Per-core packed input. Core c: t = c//2, n-half h = c%2.

    xt is host-rotated so the core's own 2048 dest columns come first
    (a rotation does not change a sum over all source nodes).
    """
    import ml_dtypes

    bf16 = ml_dtypes.bfloat16
    fp8 = ml_dtypes.float8_e4m3
    x = np.asarray(x, dtype=np.float32)
    W = np.ascontiguousarray(np.asarray(W, dtype=np.float32))
    WT = np.ascontiguousarray(W.T)
    av = np.ascontiguousarray(
        np.stack([np.asarray(a1, np.float32), np.asarray(a2, np.float32)], axis=1)
    )
    prm = np.ascontiguousarray(np.concatenate([W, WT, av], axis=1).astype(bf16))
    in_maps = []
    for c in range(8):
        t, h = c // 2, c % 2
        xt = x[:, t, :].T  # [D, N]
        if h == 1:
            xt = np.concatenate([xt[:, HALF:], xt[:, :HALF]], axis=1)
        xn = x[h * HALF : (h + 1) * HALF, t, :]  # [2048, 128]
        xn_packed = np.ascontiguousarray(
            xn.reshape(NT, P, D).transpose(1, 0, 2).reshape(P, NT * D)
        )
        in_maps.append(
            {
                "prm": prm,
                "xt": np.ascontiguousarray(xt.astype(fp8)),
                "xn": xn_packed,
            }
        )
    return in_maps


def _run(x, W, a1, a2, trace=False):
    from concourse.bass_utils import run_bass_kernel_spmd

    key = "nc"
    if key not in _CACHE:
        _CACHE[key] = _build()
    nc = _CACHE[key]
    in_maps = _prep_inputs(x, W, a1, a2)
    res = run_bass_kernel_spmd(nc, in_maps, list(range(8)), trace=trace)
    out_full = np.empty((N, T, D), dtype=np.float32)
    for c in range(8):
        t, h = c // 2, c % 2
        o = res.results[c]["out"].reshape(P, NT, D).transpose(1, 0, 2)
        out_full[h * HALF : (h + 1) * HALF, t, :] = o.reshape(HALF, D)
    return out_full, res


def kernel(x, W, a1, a2):
    out, _ = _run(x, W, a1, a2, trace=False)
    return out
